# revision 1
# baseline (speedup 1.0000x reference)
"""Trainium2 Bass kernel for nn_Cross_SelfAttention (B=2, C=256, H=W=64, DQ=16).

Sharding: 8 cores = (batch b in {0,1}) x (attn stream s in {0,1}) x
(query half h in {0,1}).  Each core computes, for its (b, s):
    q = Wq @ x_s + bq   (only its query half i)
    k = Wk @ x_s        (bk dropped: constant-in-j terms cancel in softmax)
    S^T[j, i] = k[:, j] . q[:, i]
    E = exp(S^T) (no max subtraction; |S| <= ~15 so fp32/bf16 exp is safe)
    acc[st] = V_st^T-weighted sums of E columns (st = v1/v2 x 2 c-chunks)
    rowsum  = ones-stationary matmul over the same E
    o = (gamma*Wpt) @ ocat + bpt_eff  (bias via k=1 ones-row matmul)
    out = o * recip(rowsum) + x_residual
bv is folded into bpt_eff on the host (normalization makes the missing
V-bias contribution exactly Wpt @ [bv; bv]); gamma is folded into Wpt/bpt.

Each core writes a disjoint [256, 2048] slice of the output; no
collectives needed.
"""

import os

import numpy as np
import ml_dtypes

import concourse.bass as bass
import concourse.bacc as bacc
import concourse.mybir as mybir
from concourse.tile import TileContext
from concourse.bass import ts

BF16 = mybir.dt.bfloat16
F32 = mybir.dt.float32
F32R = mybir.dt.float32r

def _r(ap):
    """View an fp32 AP as float32r for full-rate PE matmuls (N>=256)."""
    return ap.bitcast(F32R)

B, C, HW, DQ = 2, 256, 4096, 16
HALF = HW // 2          # query positions per core
IB = 512                # i-block size (one PSUM bank at fp32)
N_IB = HALF // IB       # 4 i-blocks
N_JC = HW // 128        # 32 j-chunks

_NC_CACHE = {}

# Debug knob: repeat the main attention loop KREP times inside the program
# (device-time slope measurement through constant dispatch overhead).
KREP = int(os.environ.get("KREP", "1"))


def build_bass(krep=None):
    krep = KREP if krep is None else krep
    if krep in _NC_CACHE:
        return _NC_CACHE[krep]

    nc = bacc.Bacc("TRN2", target_bir_lowering=False, debug=False, num_devices=8)

    # Per-core inputs (full K/V range, query-half for q/residual).
    xq32_d = nc.dram_tensor("xq32", [C, HALF], F32, kind="ExternalInput")
    xk_d = nc.dram_tensor("xk32", [C, HW], F32R, kind="ExternalInput")
    # x streams pre-transposed on host: [HW, C], j on partitions after tiling
    xv1_d = nc.dram_tensor("xv1T", [HW, C], BF16, kind="ExternalInput")
    xv2_d = nc.dram_tensor("xv2T", [HW, C], BF16, kind="ExternalInput")
    # wq/wk replicated twice along M (cols 0:16 and 32:48) so S^T can use
    # 2x tile_position row-packing (contraction is only DQ=16 deep).
    wq_d = nc.dram_tensor("wqT", [C, 48], F32, kind="ExternalInput")
    wk_d = nc.dram_tensor("wkT", [C, 48], F32R, kind="ExternalInput")
    # wcat[r*C + c', c] = (gamma * Wpt[:, r-block] @ Wv)[c, c'] pre-composed
    # on host — the Wv projection and the output 1x1 conv fused into one.
    wcat_d = nc.dram_tensor("wcat", [2 * C, C], F32R, kind="ExternalInput")
    bq_d = nc.dram_tensor("bq_row", [1, 48], F32, kind="ExternalInput")
    bpt_d = nc.dram_tensor("bpt_col", [128, 2], F32, kind="ExternalInput")
    out_d = nc.dram_tensor("out", [C, HALF], F32, kind="ExternalOutput")

    with TileContext(nc) as tc:
        with (
            tc.tile_pool(name="persist", bufs=1) as pp,
            tc.tile_pool(name="work", bufs=1) as wp,
            tc.tile_pool(name="psum", bufs=1, space="PSUM") as psp,
        ):
            # ---- persistent SBUF tensors ----
            xq32 = pp.tile([128, 2, HALF], F32, name="xq32_sb")
            xk = pp.tile([128, 2, HW], F32R, name="xk_sb")
            xvt = [
                pp.tile([128, N_JC, C], BF16, name=f"xvt{r}_sb", tag=f"xvt{r}")
                for r in range(2)
            ]
            wq = pp.tile([128, 2, 48], F32, name="wq_sb")
            wk = pp.tile([128, 2, 48], F32R, name="wk_sb")
            wcat = pp.tile([128, 4, C], F32R, name="wcat_sb")
            bq = pp.tile([1, 48], F32, name="bq_sb")
            bpt = pp.tile([128, 2], F32, name="bpt_sb")
            ones_row = pp.tile([1, IB], F32, name="ones_row")
            ones128 = pp.tile([128, 128], BF16, name="ones128")
            qsb = pp.tile([48, HALF], F32R, name="qsb")
            ksb = pp.tile([48, HW], F32R, name="ksb")

            nc.vector.memset(ones_row[:], 1.0)
            nc.vector.memset(ones128[:], 1.0)

            # ---- load weights + x ----
            r128 = lambda ap: ap.rearrange("(o p) f -> p o f", p=128)
            nc.sync.dma_start(wq[:], r128(wq_d))
            nc.sync.dma_start(wk[:], r128(wk_d))
            nc.sync.dma_start(wcat[:], r128(wcat_d))
            nc.sync.dma_start(bq[:], bq_d[:])
            nc.sync.dma_start(bpt[:], bpt_d[:])
            for _rep in range(krep):
                nc.sync.dma_start(xq32[:], r128(xq32_d))
                nc.sync.dma_start(xk[:], r128(xk_d))
                nc.sync.dma_start(xvt[0][:], r128(xv1_d))
                nc.sync.dma_start(xvt[1][:], r128(xv2_d))

                # ---- Q projection (with bias), K projection (no bias) ----
                for p4 in range(N_IB):
                    q_ps = psp.tile([128, IB], F32, name="q_ps", tag="s", bufs=2)
                    nc.tensor.matmul(
                        q_ps[:48], wq[:, 0], xq32[:, 0, ts(p4, IB)],
                        start=True, stop=False,
                    )
                    nc.tensor.matmul(
                        q_ps[:48], wq[:, 1], xq32[:, 1, ts(p4, IB)],
                        start=False, stop=False,
                    )
                    nc.tensor.matmul(
                        q_ps[:48], bq[:], ones_row[:], start=False, stop=True,
                    )
                    nc.vector.tensor_copy(qsb[:, ts(p4, IB)], q_ps[:48])
                for p8 in range(HW // IB):
                    k_ps = psp.tile([128, IB], F32, name="k_ps", tag="s", bufs=2)
                    nc.tensor.matmul(
                        k_ps[:48], wk[:, 0], xk[:, 0, ts(p8, IB)],
                        start=True, stop=False,
                    )
                    nc.tensor.matmul(
                        k_ps[:48], wk[:, 1], xk[:, 1, ts(p8, IB)],
                        start=False, stop=True,
                    )
                    nc.vector.tensor_copy(ksb[:, ts(p8, IB)], k_ps[:48])

                # ---- main attention loop over i-blocks ----
                for ib in range(N_IB):
                    accs = [
                        psp.tile([128, IB], F32, name=f"acc{st}", tag="acc", bufs=5)
                        for st in range(4)
                    ]
                    acc1 = psp.tile([128, IB], F32, name="acc_ones", tag="acc", bufs=5)
                    for jp in range(N_JC // 2):
                        # two S^T chunks concurrently in PE row-groups 0 / 1
                        s_a = psp.tile([128, IB], F32, name="s_a", tag="s", bufs=2)
                        s_b = psp.tile([128, IB], F32, name="s_b", tag="s", bufs=2)
                        nc.tensor.matmul(
                            s_a[:], ksb[0:16, ts(2 * jp, 128)],
                            qsb[0:16, ts(ib, IB)],
                            start=True, stop=True, tile_position=(0, 0),
                        )
                        nc.tensor.matmul(
                            s_b[:], ksb[32:48, ts(2 * jp + 1, 128)],
                            qsb[32:48, ts(ib, IB)],
                            start=True, stop=True, tile_position=(32, 0),
                        )
                        for jc, s_ps in ((2 * jp, s_a), (2 * jp + 1, s_b)):
                            e_t = wp.tile([128, IB], BF16, name="e_t", tag="E", bufs=3)
                            nc.scalar.activation(
                                e_t[:], s_ps[:], mybir.ActivationFunctionType.Exp
                            )
                            for st in range(4):
                                nc.tensor.matmul(
                                    accs[st][:],
                                    xvt[st // 2][:, jc, ts(st % 2, 128)],
                                    e_t[:],
                                    start=(jc == 0), stop=(jc == N_JC - 1),
                                )
                            nc.tensor.matmul(
                                acc1[:], ones128[:], e_t[:],
                                start=(jc == 0), stop=(jc == N_JC - 1),
                            )

                    r_t = wp.tile([128, IB], F32, name="r_t", tag="R", bufs=2)
                    nc.vector.reciprocal(r_t[:], acc1[:])
                    ocat = wp.tile([128, 4, IB], F32R, name="ocat", tag="ocat", bufs=2)
                    for st in range(4):
                        nc.vector.tensor_copy(ocat[:, st], accs[st][:])

                    for cc in range(2):
                        p_ps = psp.tile([128, IB], F32, name="p_ps", tag="proj", bufs=1)
                        for cp in range(4):
                            nc.tensor.matmul(
                                p_ps[:], wcat[:, cp, ts(cc, 128)], ocat[:, cp],
                                start=(cp == 0), stop=(cp == 3),
                            )
                        o_t = wp.tile([128, IB], F32, name="o_t", tag="osb", bufs=3)
                        nc.vector.tensor_mul(o_t[:], p_ps[:], r_t[:])
                        # (o + bpt_eff) + x_residual; bpt is a per-partition scalar
                        nc.vector.scalar_tensor_tensor(
                            o_t[:], o_t[:], bpt[:, cc:cc + 1], xq32[:, cc, ts(ib, IB)],
                            op0=mybir.AluOpType.add, op1=mybir.AluOpType.add,
                        )
                        nc.sync.dma_start(
                            out_d.rearrange("(o p) f -> p o f", p=128)[:, cc, ts(ib, IB)],
                            o_t[:],
                        )

    nc.compile()
    _NC_CACHE[krep] = nc
    return nc


def _prep_maps(x, Wq, bq, Wk, bk, Wv, bv, Wpt, bpt, gamma):
    bf16 = ml_dtypes.bfloat16
    f32 = np.float32
    g = float(np.asarray(gamma).reshape(-1)[0])
    # wq/wk/bq replicated at column offsets 0 and 32 (S^T 2x row-packing)
    wqT = np.zeros((C, 48), f32)
    wqT[:, 0:DQ] = Wq.T
    wqT[:, 32:32 + DQ] = Wq.T
    wkT = np.zeros((C, 48), f32)
    wkT[:, 0:DQ] = Wk.T
    wkT[:, 32:32 + DQ] = Wk.T
    bq_row = np.zeros((1, 48), f32)
    bq_row[0, 0:DQ] = bq
    bq_row[0, 32:32 + DQ] = bq
    # fuse the Wv projection into the output 1x1 conv:
    #   o = sum_r (g*Wpt[:, r-block] @ Wv) @ (X_r E) ;  wcat rows = c' of X_r
    wpt_g = (g * Wpt).astype(f32)
    wcat = np.concatenate(
        [(wpt_g[:, :C] @ Wv).T, (wpt_g[:, C:] @ Wv).T], axis=0
    ).astype(f32)  # [2C, C]: row r*C+c', col c
    bpt_eff = (g * (bpt + Wpt @ np.concatenate([bv, bv]))).astype(np.float32)
    bpt_col = np.ascontiguousarray(bpt_eff.reshape(2, 128).T)

    xf = np.asarray(x, np.float32).reshape(B, 2, C, HW)
    in_maps = []
    for core in range(8):
        b, s, h = core >> 2, (core >> 1) & 1, core & 1
        in_maps.append(
            dict(
                xq32=np.ascontiguousarray(xf[b, s, :, h * HALF:(h + 1) * HALF]),
                xk32=np.ascontiguousarray(xf[b, s]),
                xv1T=np.ascontiguousarray(xf[b, 0].T.astype(bf16)),
                xv2T=np.ascontiguousarray(xf[b, 1].T.astype(bf16)),
                wqT=wqT, wkT=wkT, wcat=wcat,
                bq_row=bq_row, bpt_col=bpt_col,
            )
        )
    return in_maps


def kernel(x, Wq, bq, Wk, bk, Wv, bv, Wpt, bpt, gamma, _trace=False):
    from concourse.bass_utils import run_bass_kernel_spmd

    nc = build_bass()
    in_maps = _prep_maps(x, Wq, bq, Wk, bk, Wv, bv, Wpt, bpt, gamma)
    res = run_bass_kernel_spmd(nc, in_maps, list(range(8)), trace=_trace)

    out = np.empty((B, 2, C, HW), np.float32)
    for core in range(8):
        b, s, h = core >> 2, (core >> 1) & 1, core & 1
        out[b, s, :, h * HALF:(h + 1) * HALF] = res.results[core]["out"]
    full = out.reshape(B, 2 * C, 64, 64)
    if _trace:
        return full, res
    return full



# revision 7
# speedup vs baseline: 1.2146x; 1.2146x over previous
"""Trainium2 Bass kernel for nn_Cross_SelfAttention (B=2, C=256, H=W=64, DQ=16).

Sharding: 8 cores = (batch b in {0,1}) x (attn stream s in {0,1}) x
(query half h in {0,1}).  Each core computes, for its (b, s):
    q = Wq @ x_s + bq   (only its query half i)
    k = Wk @ x_s        (bk dropped: constant-in-j terms cancel in softmax)
    S^T[j, i] = k[:, j] . q[:, i]
    E = exp(S^T) (no max subtraction; |S| <= ~15 so fp32/bf16 exp is safe)
    acc[st] = V_st^T-weighted sums of E columns (st = v1/v2 x 2 c-chunks)
    rowsum  = DVE-accumulated E chunks + one ones-matmul per i-block
    o = (gamma*Wpt) @ ocat + bpt_eff
    out = o * recip(rowsum) + x_residual
bv is folded into bpt_eff on the host; gamma is folded into Wpt/bpt.

PE budget per core: V.E accs dominate (4 ib x 32 jc x 4 x 512 cyc);
S^T is 3-way row-group packed (K=16 only); rowsum and q-bias are off
the PE entirely.

Each core writes a disjoint [256, 2048] slice of the output; no
collectives needed.
"""

import os

import numpy as np
import ml_dtypes

import concourse.bass as bass
import concourse.bacc as bacc
import concourse.mybir as mybir
from concourse.tile import TileContext
from concourse.bass import ts

BF16 = mybir.dt.bfloat16
F32 = mybir.dt.float32
F32R = mybir.dt.float32r

def _r(ap):
    """View an fp32 AP as float32r for full-rate PE matmuls (N>=256)."""
    return ap.bitcast(F32R)

B, C, HW, DQ = 2, 256, 4096, 16
HALF = HW // 2          # query positions per core
IB = 512                # i-block size (one PSUM bank at fp32)
N_IB = HALF // IB       # 4 i-blocks
N_JC = HW // 128        # 32 j-chunks
NREP = 3                # S^T row-group packing degree (replicas at 0/32/64)
WREP = 32 * (NREP - 1) + DQ  # 80 partitions for replicated q/k

_NC_CACHE = {}

# Debug knob: repeat the main attention loop KREP times inside the program
# (device-time slope measurement through constant dispatch overhead).
KREP = int(os.environ.get("KREP", "1"))


def build_bass(krep=None):
    krep = KREP if krep is None else krep
    if krep in _NC_CACHE:
        return _NC_CACHE[krep]

    nc = bacc.Bacc("TRN2", target_bir_lowering=False, debug=False, num_devices=8)

    # Per-core inputs (full K/V range, query-half for q/residual).
    xq32_d = nc.dram_tensor("xq32", [C, HALF], F32R, kind="ExternalInput")
    xk_d = nc.dram_tensor("xk32", [C, HW], F32R, kind="ExternalInput")
    # x streams pre-transposed on host: [HW, C], j on partitions after tiling
    xv1_d = nc.dram_tensor("xv1T", [HW, C], BF16, kind="ExternalInput")
    xv2_d = nc.dram_tensor("xv2T", [HW, C], BF16, kind="ExternalInput")
    # wq/wk replicated 3x along M (cols 0:16, 32:48, 64:80) so S^T can use
    # 3x tile_position row-packing (contraction is only DQ=16 deep).
    wq_d = nc.dram_tensor("wqT", [C, WREP], F32R, kind="ExternalInput")
    wk_d = nc.dram_tensor("wkT", [C, WREP], F32R, kind="ExternalInput")
    # wcat[r*C + c', c] = (gamma * Wpt[:, r-block] @ Wv)[c, c'] pre-composed
    # on host — the Wv projection and the output 1x1 conv fused into one.
    wcat_d = nc.dram_tensor("wcat", [2 * C, C], F32R, kind="ExternalInput")
    bq_d = nc.dram_tensor("bq_col", [WREP, 1], F32, kind="ExternalInput")
    bpt_d = nc.dram_tensor("bpt_col", [128, 2], F32, kind="ExternalInput")
    out_d = nc.dram_tensor("out", [C, HALF], F32, kind="ExternalOutput")

    with TileContext(nc) as tc:
        with (
            tc.tile_pool(name="persist", bufs=1) as pp,
            tc.tile_pool(name="work", bufs=1) as wp,
            tc.tile_pool(name="psum", bufs=1, space="PSUM") as psp,
        ):
            # ---- persistent SBUF tensors ----
            xq32 = pp.tile([128, 2, HALF], F32R, name="xq32_sb")
            xk = pp.tile([128, 2, HW], F32R, name="xk_sb")
            xvt = [
                pp.tile([128, N_JC, C], BF16, name=f"xvt{r}_sb", tag=f"xvt{r}")
                for r in range(2)
            ]
            wq = pp.tile([128, 2, WREP], F32R, name="wq_sb")
            wk = pp.tile([128, 2, WREP], F32R, name="wk_sb")
            wcat = pp.tile([128, 4, C], F32R, name="wcat_sb")
            bqc = pp.tile([WREP, 1], F32, name="bq_sb")
            bpt = pp.tile([128, 2], F32, name="bpt_sb")
            ones128 = pp.tile([128, 128], F32, name="ones128")
            qsb = pp.tile([WREP, HALF], F32R, name="qsb")
            ksb = pp.tile([WREP, HW], F32R, name="ksb")

            nc.vector.memset(ones128[:], 1.0)

            # ---- load weights + x (x chunked so compute can start early) ----
            r128 = lambda ap: ap.rearrange("(o p) f -> p o f", p=128)
            nc.sync.dma_start(wq[:], r128(wq_d))
            nc.sync.dma_start(wk[:], r128(wk_d))
            nc.sync.dma_start(wcat[:], r128(wcat_d))
            nc.sync.dma_start(bqc[:], bq_d[:])
            nc.sync.dma_start(bpt[:], bpt_d[:])
            xkr = r128(xk_d)
            xvr = [r128(xv1_d), r128(xv2_d)]
            for _rep in range(krep):
                nc.sync.dma_start(xq32[:, :, 0:HALF // 2],
                                  r128(xq32_d)[:, :, 0:HALF // 2])
                nc.sync.dma_start(xq32[:, :, HALF // 2:HALF],
                                  r128(xq32_d)[:, :, HALF // 2:HALF])
                for p in range(4):
                    nc.sync.dma_start(xk[:, :, ts(p, HW // 4)],
                                      xkr[:, :, ts(p, HW // 4)])
                for p in range(4):
                    for r in range(2):
                        nc.sync.dma_start(xvt[r][:, ts(p, N_JC // 4)],
                                          xvr[r][:, ts(p, N_JC // 4)])

                # ---- Q projection (bias folded into the PSUM->SBUF copy),
                # ---- K projection (no bias) ----
                for p4 in range(N_IB):
                    q_ps = psp.tile([128, IB], F32, name="q_ps", tag="s", bufs=NREP)
                    nc.tensor.matmul(
                        q_ps[:WREP], wq[:, 0], xq32[:, 0, ts(p4, IB)],
                        start=True, stop=False,
                    )
                    nc.tensor.matmul(
                        q_ps[:WREP], wq[:, 1], xq32[:, 1, ts(p4, IB)],
                        start=False, stop=True,
                    )
                    nc.vector.tensor_scalar_add(qsb[:, ts(p4, IB)], q_ps[:WREP], bqc[:])
                for p8 in range(HW // IB):
                    k_ps = psp.tile([128, IB], F32, name="k_ps", tag="s", bufs=NREP)
                    nc.tensor.matmul(
                        k_ps[:WREP], wk[:, 0], xk[:, 0, ts(p8, IB)],
                        start=True, stop=False,
                    )
                    nc.tensor.matmul(
                        k_ps[:WREP], wk[:, 1], xk[:, 1, ts(p8, IB)],
                        start=False, stop=True,
                    )
                    nc.vector.tensor_copy(ksb[:, ts(p8, IB)], k_ps[:WREP])

                # ---- main attention loop over i-blocks ----
                n_jg = (N_JC + NREP - 1) // NREP
                for ib in range(N_IB):
                    accs = [
                        psp.tile([128, IB], F32, name=f"acc{st}", tag="acc", bufs=4)
                        for st in range(4)
                    ]
                    racc = wp.tile([128, IB], F32, name="racc", tag="racc", bufs=1)
                    for jg in range(n_jg):
                        jcs = list(range(NREP * jg, min(NREP * jg + NREP, N_JC)))
                        s_ps = [
                            psp.tile([128, IB], F32, name=f"s{t}", tag="s", bufs=NREP)
                            for t in range(len(jcs))
                        ]
                        for t, jc in enumerate(jcs):
                            nc.tensor.matmul(
                                s_ps[t], ksb[32 * t:32 * t + DQ, ts(jc, 128)],
                                qsb[32 * t:32 * t + DQ, ts(ib, IB)],
                                start=True, stop=True, tile_position=(32 * t, 0),
                            )
                        for t, jc in enumerate(jcs):
                            e_t = wp.tile([128, IB], BF16, name="e_t", tag="E", bufs=3)
                            nc.scalar.activation(
                                e_t[:], s_ps[t][:], mybir.ActivationFunctionType.Exp
                            )
                            for st in range(4):
                                nc.tensor.matmul(
                                    accs[st][:],
                                    xvt[st // 2][:, jc, ts(st % 2, 128)],
                                    e_t[:],
                                    start=(jc == 0), stop=(jc == N_JC - 1),
                                )
                            if jc == 0:
                                nc.vector.tensor_copy(racc[:], e_t[:])
                            else:
                                nc.vector.tensor_add(racc[:], racc[:], e_t[:])

                    racc_ps = psp.tile([128, IB], F32, name="racc_ps", tag="racc_ps",
                                       bufs=1)
                    nc.tensor.matmul(
                        racc_ps[:], ones128[:], racc[:], start=True, stop=True,
                    )
                    r_t = wp.tile([128, IB], F32, name="r_t", tag="R", bufs=2)
                    nc.vector.reciprocal(r_t[:], racc_ps[:])
                    ocat = wp.tile([128, 4, IB], F32R, name="ocat", tag="ocat", bufs=2)
                    for st in range(4):
                        nc.vector.tensor_copy(ocat[:, st], accs[st][:])

                    for cc in range(2):
                        p_ps = psp.tile([128, IB], F32, name="p_ps", tag="s", bufs=NREP)
                        for cp in range(4):
                            nc.tensor.matmul(
                                p_ps[:], wcat[:, cp, ts(cc, 128)], ocat[:, cp],
                                start=(cp == 0), stop=(cp == 3),
                            )
                        o_t = wp.tile([128, IB], F32, name="o_t", tag="osb", bufs=3)
                        nc.vector.tensor_mul(o_t[:], p_ps[:], r_t[:])
                        # (o + bpt_eff) + x_residual; bpt is a per-partition scalar
                        nc.vector.scalar_tensor_tensor(
                            o_t[:], o_t[:], bpt[:, cc:cc + 1],
                            xq32[:, cc, ts(ib, IB)].bitcast(F32),
                            op0=mybir.AluOpType.add, op1=mybir.AluOpType.add,
                        )
                        nc.sync.dma_start(
                            out_d.rearrange("(o p) f -> p o f", p=128)[:, cc, ts(ib, IB)],
                            o_t[:],
                        )

    nc.compile()
    _NC_CACHE[krep] = nc
    return nc


def _prep_maps(x, Wq, bq, Wk, bk, Wv, bv, Wpt, bpt, gamma):
    bf16 = ml_dtypes.bfloat16
    f32 = np.float32
    g = float(np.asarray(gamma).reshape(-1)[0])
    # wq/wk/bq replicated at column offsets 0, 32, 64 (S^T 3x row-packing)
    wqT = np.zeros((C, WREP), f32)
    wkT = np.zeros((C, WREP), f32)
    bq_col = np.zeros((WREP, 1), f32)
    for t in range(NREP):
        wqT[:, 32 * t:32 * t + DQ] = Wq.T
        wkT[:, 32 * t:32 * t + DQ] = Wk.T
        bq_col[32 * t:32 * t + DQ, 0] = bq
    # fuse the Wv projection into the output 1x1 conv:
    #   o = sum_r (g*Wpt[:, r-block] @ Wv) @ (X_r E) ;  wcat rows = c' of X_r
    wpt_g = (g * Wpt).astype(f32)
    wcat = np.concatenate(
        [(wpt_g[:, :C] @ Wv).T, (wpt_g[:, C:] @ Wv).T], axis=0
    ).astype(f32)  # [2C, C]: row r*C+c', col c
    bpt_eff = (g * (bpt + Wpt @ np.concatenate([bv, bv]))).astype(np.float32)
    bpt_col = np.ascontiguousarray(bpt_eff.reshape(2, 128).T)

    xf = np.asarray(x, np.float32).reshape(B, 2, C, HW)
    in_maps = []
    for core in range(8):
        b, s, h = core >> 2, (core >> 1) & 1, core & 1
        in_maps.append(
            dict(
                xq32=np.ascontiguousarray(xf[b, s, :, h * HALF:(h + 1) * HALF]),
                xk32=np.ascontiguousarray(xf[b, s]),
                xv1T=np.ascontiguousarray(xf[b, 0].T.astype(bf16)),
                xv2T=np.ascontiguousarray(xf[b, 1].T.astype(bf16)),
                wqT=wqT, wkT=wkT, wcat=wcat,
                bq_col=bq_col, bpt_col=bpt_col,
            )
        )
    return in_maps


def kernel(x, Wq, bq, Wk, bk, Wv, bv, Wpt, bpt, gamma, _trace=False):
    from concourse.bass_utils import run_bass_kernel_spmd

    nc = build_bass()
    in_maps = _prep_maps(x, Wq, bq, Wk, bk, Wv, bv, Wpt, bpt, gamma)
    res = run_bass_kernel_spmd(nc, in_maps, list(range(8)), trace=_trace)

    out = np.empty((B, 2, C, HW), np.float32)
    for core in range(8):
        b, s, h = core >> 2, (core >> 1) & 1, core & 1
        out[b, s, :, h * HALF:(h + 1) * HALF] = res.results[core]["out"]
    full = out.reshape(B, 2 * C, 64, 64)
    if _trace:
        return full, res
    return full


# revision 8
# speedup vs baseline: 1.6413x; 1.3513x over previous
"""Trainium2 Bass kernel for nn_Cross_SelfAttention (B=2, C=256, H=W=64, DQ=16).

Sharding: 8 cores = (batch b in {0,1}) x (attn stream s in {0,1}) x
(query half h in {0,1}).  Each core computes, for its (b, s):
    q = Wq @ x_s + bq   (only its query half i)
    k = Wk @ x_s        (bk dropped: constant-in-j terms cancel in softmax)
    S^T[j, i] = k[:, j] . q[:, i]
    E = exp(S^T) (no max subtraction; |S| <= ~15 so fp32/bf16 exp is safe)
    acc[st] = V_st^T-weighted sums of E columns (st = v1/v2 x 2 c-chunks)
    rowsum  = DVE-accumulated E chunks + one ones-matmul per i-block
    o = (gamma*Wpt) @ ocat + bpt_eff
    out = o * recip(rowsum) + x_residual
bv is folded into bpt_eff on the host; gamma is folded into Wpt/bpt.

PE budget per core: V.E accs dominate (4 ib x 32 jc x 4 x 512 cyc);
S^T is 3-way row-group packed (K=16 only); rowsum and q-bias are off
the PE entirely.

Each core writes a disjoint [256, 2048] slice of the output; no
collectives needed.
"""

import os

import numpy as np
import ml_dtypes

import concourse.bass as bass
import concourse.bacc as bacc
import concourse.mybir as mybir
from concourse.tile import TileContext
from concourse.bass import ts

BF16 = mybir.dt.bfloat16
F32 = mybir.dt.float32
F32R = mybir.dt.float32r
F8E4 = mybir.dt.float8e4

def _r(ap):
    """View an fp32 AP as float32r for full-rate PE matmuls (N>=256)."""
    return ap.bitcast(F32R)

B, C, HW, DQ = 2, 256, 4096, 16
HALF = HW // 2          # query positions per core
IB = 512                # i-block size (one PSUM bank at fp32)
N_IB = HALF // IB       # 4 i-blocks
N_JC = HW // 128        # 32 j-chunks
NREP = 3
E8SCALE = 192.0     # fp8 softmax-weight scale (TRN e4m3 max is 240)                # S^T row-group packing degree (replicas at 0/32/64)
WREP = 32 * (NREP - 1) + DQ  # 80 partitions for replicated q/k

_NC_CACHE = {}

# Debug knob: repeat the main attention loop KREP times inside the program
# (device-time slope measurement through constant dispatch overhead).
KREP = int(os.environ.get("KREP", "1"))


def build_bass(krep=None):
    krep = KREP if krep is None else krep
    if krep in _NC_CACHE:
        return _NC_CACHE[krep]

    nc = bacc.Bacc("TRN2", target_bir_lowering=False, debug=False, num_devices=8)

    # Per-core inputs (full K/V range, query-half for q/residual).
    xq32_d = nc.dram_tensor("xq32", [C, HALF], F32R, kind="ExternalInput")
    xk_d = nc.dram_tensor("xk32", [C, HW], F32R, kind="ExternalInput")
    # x streams pre-transposed on host: [HW, C], j on partitions after tiling
    xv1_d = nc.dram_tensor("xv1T", [HW, C], F8E4, kind="ExternalInput")
    xv2_d = nc.dram_tensor("xv2T", [HW, C], F8E4, kind="ExternalInput")
    # wq/wk replicated 3x along M (cols 0:16, 32:48, 64:80) so S^T can use
    # 3x tile_position row-packing (contraction is only DQ=16 deep).
    wq_d = nc.dram_tensor("wqT", [C, WREP], F32R, kind="ExternalInput")
    wk_d = nc.dram_tensor("wkT", [C, WREP], F32R, kind="ExternalInput")
    # wcat[r*C + c', c] = (gamma * Wpt[:, r-block] @ Wv)[c, c'] pre-composed
    # on host — the Wv projection and the output 1x1 conv fused into one.
    wcat_d = nc.dram_tensor("wcat", [2 * C, C], F32R, kind="ExternalInput")
    bq_d = nc.dram_tensor("bq_col", [WREP, 1], F32, kind="ExternalInput")
    bpt_d = nc.dram_tensor("bpt_col", [128, 2], F32, kind="ExternalInput")
    out_d = nc.dram_tensor("out", [C, HALF], F32, kind="ExternalOutput")

    with TileContext(nc) as tc:
        with (
            tc.tile_pool(name="persist", bufs=1) as pp,
            tc.tile_pool(name="work", bufs=1) as wp,
            tc.tile_pool(name="psum", bufs=1, space="PSUM") as psp,
        ):
            # ---- persistent SBUF tensors ----
            xq32 = pp.tile([128, 2, HALF], F32R, name="xq32_sb")
            xk = pp.tile([128, 2, HW], F32R, name="xk_sb")
            xvt = [
                pp.tile([128, N_JC, C], F8E4, name=f"xvt{r}_sb", tag=f"xvt{r}")
                for r in range(2)
            ]
            wq = pp.tile([128, 2, WREP], F32R, name="wq_sb")
            wk = pp.tile([128, 2, WREP], F32R, name="wk_sb")
            wcat = pp.tile([128, 4, C], F32R, name="wcat_sb")
            bqc = pp.tile([WREP, 1], F32, name="bq_sb")
            bpt = pp.tile([128, 2], F32, name="bpt_sb")
            ones128 = pp.tile([128, 128], F32, name="ones128")
            qsb = pp.tile([WREP, HALF], F32R, name="qsb")
            ksb = pp.tile([WREP, HW], F32R, name="ksb")

            nc.vector.memset(ones128[:], 1.0 / E8SCALE)

            # ---- load weights + x (x chunked so compute can start early) ----
            r128 = lambda ap: ap.rearrange("(o p) f -> p o f", p=128)
            nc.sync.dma_start(wq[:], r128(wq_d))
            nc.sync.dma_start(wk[:], r128(wk_d))
            nc.sync.dma_start(wcat[:], r128(wcat_d))
            nc.sync.dma_start(bqc[:], bq_d[:])
            nc.sync.dma_start(bpt[:], bpt_d[:])
            xkr = r128(xk_d)
            xvr = [r128(xv1_d), r128(xv2_d)]
            for _rep in range(krep):
                nc.sync.dma_start(xq32[:, :, 0:HALF // 2],
                                  r128(xq32_d)[:, :, 0:HALF // 2])
                nc.sync.dma_start(xq32[:, :, HALF // 2:HALF],
                                  r128(xq32_d)[:, :, HALF // 2:HALF])
                for p in range(4):
                    nc.sync.dma_start(xk[:, :, ts(p, HW // 4)],
                                      xkr[:, :, ts(p, HW // 4)])
                for p in range(4):
                    for r in range(2):
                        nc.sync.dma_start(xvt[r][:, ts(p, N_JC // 4)],
                                          xvr[r][:, ts(p, N_JC // 4)])

                # ---- Q projection (bias folded into the PSUM->SBUF copy),
                # ---- K projection (no bias) ----
                for p4 in range(N_IB):
                    q_ps = psp.tile([128, IB], F32, name="q_ps", tag="s", bufs=NREP)
                    nc.tensor.matmul(
                        q_ps[:WREP], wq[:, 0], xq32[:, 0, ts(p4, IB)],
                        start=True, stop=False,
                    )
                    nc.tensor.matmul(
                        q_ps[:WREP], wq[:, 1], xq32[:, 1, ts(p4, IB)],
                        start=False, stop=True,
                    )
                    nc.vector.tensor_scalar_add(qsb[:, ts(p4, IB)], q_ps[:WREP], bqc[:])
                for p8 in range(HW // IB):
                    k_ps = psp.tile([128, IB], F32, name="k_ps", tag="s", bufs=NREP)
                    nc.tensor.matmul(
                        k_ps[:WREP], wk[:, 0], xk[:, 0, ts(p8, IB)],
                        start=True, stop=False,
                    )
                    nc.tensor.matmul(
                        k_ps[:WREP], wk[:, 1], xk[:, 1, ts(p8, IB)],
                        start=False, stop=True,
                    )
                    nc.vector.tensor_copy(ksb[:, ts(p8, IB)], k_ps[:WREP])

                # ---- main attention loop over i-blocks ----
                n_jg = (N_JC + NREP - 1) // NREP
                for ib in range(N_IB):
                    accs = [
                        psp.tile([128, IB], F32, name=f"acc{st}", tag="acc", bufs=4)
                        for st in range(4)
                    ]
                    racc = wp.tile([128, IB], F32, name="racc", tag="racc", bufs=1)
                    e_sto = wp.tile([128, N_JC, IB], BF16, name="e_sto",
                                    tag="esto", bufs=1)
                    e8_sto = wp.tile([128, N_JC, IB], F8E4, name="e8_sto",
                                     tag="e8sto", bufs=1)
                    # pass 1: S^T -> exp (stored bf16), rowsum off-PE
                    for jg in range(n_jg):
                        jcs = list(range(NREP * jg, min(NREP * jg + NREP, N_JC)))
                        s_ps = [
                            psp.tile([128, IB], F32, name=f"s{t}", tag="s", bufs=NREP)
                            for t in range(len(jcs))
                        ]
                        for t, jc in enumerate(jcs):
                            nc.tensor.matmul(
                                s_ps[t], ksb[32 * t:32 * t + DQ, ts(jc, 128)],
                                qsb[32 * t:32 * t + DQ, ts(ib, IB)],
                                start=True, stop=True, tile_position=(32 * t, 0),
                            )
                        for t, jc in enumerate(jcs):
                            nc.scalar.activation(
                                e_sto[:, jc, :], s_ps[t][:],
                                mybir.ActivationFunctionType.Exp
                            )
                            if jc == 0:
                                nc.vector.tensor_copy(racc[:], e_sto[:, 0, :])
                            else:
                                nc.vector.tensor_add(racc[:], racc[:], e_sto[:, jc, :])

                    racc_ps = psp.tile([128, IB], F32, name="racc_ps", tag="racc_ps",
                                       bufs=1)
                    nc.tensor.matmul(
                        racc_ps[:], ones128[:], racc[:], start=True, stop=True,
                    )
                    r_t = wp.tile([128, IB], F32, name="r_t", tag="R", bufs=2)
                    nc.vector.reciprocal(r_t[:], racc_ps[:])

                    # pass 2: e8 = e * (E8SCALE/rowsum) -> fp8; DoubleRow V.E
                    for m in range(N_JC // 2):
                        for d in range(2):
                            jc = 2 * m + d
                            nc.vector.tensor_mul(
                                e8_sto[:, jc, :], e_sto[:, jc, :], r_t[:]
                            )
                        for st in range(4):
                            nc.tensor.matmul(
                                accs[st][:],
                                xvt[st // 2][:, 2 * m:2 * m + 2, ts(st % 2, 128)],
                                e8_sto[:, 2 * m:2 * m + 2, :],
                                start=(m == 0), stop=(m == N_JC // 2 - 1),
                                perf_mode=mybir.MatmulPerfMode.DoubleRow,
                            )

                    ocat = wp.tile([128, 4, IB], F32R, name="ocat", tag="ocat", bufs=2)
                    for st in range(4):
                        nc.vector.tensor_copy(ocat[:, st], accs[st][:])

                    for cc in range(2):
                        p_ps = psp.tile([128, IB], F32, name="p_ps", tag="s", bufs=NREP)
                        for cp in range(4):
                            nc.tensor.matmul(
                                p_ps[:], wcat[:, cp, ts(cc, 128)], ocat[:, cp],
                                start=(cp == 0), stop=(cp == 3),
                            )
                        o_t = wp.tile([128, IB], F32, name="o_t", tag="osb", bufs=3)
                        # (o + bpt_eff) + x_residual; bpt is a per-partition scalar
                        nc.vector.scalar_tensor_tensor(
                            o_t[:], p_ps[:], bpt[:, cc:cc + 1],
                            xq32[:, cc, ts(ib, IB)].bitcast(F32),
                            op0=mybir.AluOpType.add, op1=mybir.AluOpType.add,
                        )
                        nc.sync.dma_start(
                            out_d.rearrange("(o p) f -> p o f", p=128)[:, cc, ts(ib, IB)],
                            o_t[:],
                        )

    nc.compile()
    _NC_CACHE[krep] = nc
    return nc


def _prep_maps(x, Wq, bq, Wk, bk, Wv, bv, Wpt, bpt, gamma):
    bf16 = ml_dtypes.bfloat16
    f8 = ml_dtypes.float8_e4m3
    f32 = np.float32
    g = float(np.asarray(gamma).reshape(-1)[0])
    # wq/wk/bq replicated at column offsets 0, 32, 64 (S^T 3x row-packing)
    wqT = np.zeros((C, WREP), f32)
    wkT = np.zeros((C, WREP), f32)
    bq_col = np.zeros((WREP, 1), f32)
    for t in range(NREP):
        wqT[:, 32 * t:32 * t + DQ] = Wq.T
        wkT[:, 32 * t:32 * t + DQ] = Wk.T
        bq_col[32 * t:32 * t + DQ, 0] = bq
    # fuse the Wv projection into the output 1x1 conv:
    #   o = sum_r (g*Wpt[:, r-block] @ Wv) @ (X_r E) ;  wcat rows = c' of X_r
    wpt_g = (g / E8SCALE * Wpt).astype(f32)
    wcat = np.concatenate(
        [(wpt_g[:, :C] @ Wv).T, (wpt_g[:, C:] @ Wv).T], axis=0
    ).astype(f32)  # [2C, C]: row r*C+c', col c
    bpt_eff = (g * (bpt + Wpt @ np.concatenate([bv, bv]))).astype(np.float32)
    bpt_col = np.ascontiguousarray(bpt_eff.reshape(2, 128).T)

    xf = np.asarray(x, np.float32).reshape(B, 2, C, HW)
    in_maps = []
    for core in range(8):
        b, s, h = core >> 2, (core >> 1) & 1, core & 1
        in_maps.append(
            dict(
                xq32=np.ascontiguousarray(xf[b, s, :, h * HALF:(h + 1) * HALF]),
                xk32=np.ascontiguousarray(xf[b, s]),
                xv1T=np.ascontiguousarray(xf[b, 0].T.astype(f8)),
                xv2T=np.ascontiguousarray(xf[b, 1].T.astype(f8)),
                wqT=wqT, wkT=wkT, wcat=wcat,
                bq_col=bq_col, bpt_col=bpt_col,
            )
        )
    return in_maps


def kernel(x, Wq, bq, Wk, bk, Wv, bv, Wpt, bpt, gamma, _trace=False):
    from concourse.bass_utils import run_bass_kernel_spmd

    nc = build_bass()
    in_maps = _prep_maps(x, Wq, bq, Wk, bk, Wv, bv, Wpt, bpt, gamma)
    res = run_bass_kernel_spmd(nc, in_maps, list(range(8)), trace=_trace)

    out = np.empty((B, 2, C, HW), np.float32)
    for core in range(8):
        b, s, h = core >> 2, (core >> 1) & 1, core & 1
        out[b, s, :, h * HALF:(h + 1) * HALF] = res.results[core]["out"]
    full = out.reshape(B, 2 * C, 64, 64)
    if _trace:
        return full, res
    return full
